# revision 14
# baseline (speedup 1.0000x reference)
"""Trainium2 Bass kernel for nn_AVGAE (3-layer GAT variational graph
autoencoder, N=4096) on 8 NeuronCores.

Sharding: 1D row partition of the N x N attention/score matrices — core k
owns output rows [512k, 512k+512). Small per-node features are all-gathered
between layers (AllGather over internal DRAM tiles).

Key algebraic restructuring (no elementwise transcendentals over N x N):
  exp(leaky_relu(f1_i + f2_j, a)) = max(A_i*B_j, C_i*D_j)
  with A=exp(f1), B=exp(f2), C=exp(a*f1), D=exp(a*f2)
so each N x N score tile is built with vector ALU ops only (outer-product
scalar muls + max + mask mul), all bf16, then consumed directly by the
tensor engine as attention weights.  Softmax denominators come for free as
a ones-column in the attention rhs (exp(MASK_VAL) == 0 exactly in fp32, so
masked entries contribute 0 to numerator and denominator, matching the
reference softmax).

All per-node "h" quantities of layers 1/2 are linear images of layer-0
attention output, so the layer-0 attention rhs carries
[h0@W1 | h0@W2 | per-layer score vectors | ones] and hidden itself is never
materialized.  Host precomputes the folded weight matrix; the device-side
first matmul is X_own @ Wbig.

Layers 1 and 2 produce TRANSPOSED outputs (lhsT = the small rhs columns,
moving operand = the P tile, N=512): one matmul per (j-tile, layer) instead
of four, one PSUM bank each, and Z is produced directly in [H2, node]
layout for the fp16 Z Z^T decoder (no transposes).

Engine balance per j-tile (tuned from perfetto traces): a custom fused DVE
op RK1MAX (out = max(in0*s0, in1*s1)) builds the score tile in one VectorE
instruction for ~5/9 of tiles; the rest use two ScalarE copy-with-scale ops
plus a VectorE max.  The mask multiply runs as one VectorE op per 8 j-tiles
([128, 8*512] supertile).  GpSimd is deliberately unused for elementwise
work: its ops are ~8x slower and its SBUF-port contention slows concurrent
VectorE ops ~4x.  Short bursts of dummy fp32 matmuls (gated on gathered
data) keep the PE clock-gate (HAM) at 2.4 GHz through the VE-paced loops.
"""

import numpy as np
import ml_dtypes

import concourse.bass as bass
import concourse.mybir as mybir
import concourse.tile as tile
from concourse import bacc
from concourse.bass import ts
from concourse.bass_utils import run_bass_kernel_spmd
from concourse.masks import make_identity

import concourse.dve_ops as _dve_ops
from concourse.dve_spec import Spec as _Spec, Src0 as _Src0, Src1 as _Src1, \
    C0 as _C0, C1 as _C1, maxx as _maxx, lower as _dve_lower
from concourse.dve_uop import DveOpSpec as _DveOpSpec


def _register_rk1max():
    """Custom fused DVE op: out = max(in0*s0, in1*s1) — builds an attention
    P-tile precursor in one VectorE instruction instead of
    (scalar-mul + scalar_tensor_tensor)."""
    name = "RK1MAX"
    if name in _dve_ops._SUB_OPCODE_FOR_NAME:
        return next(o for o in _dve_ops.OPS if o.name == name)
    spec = _Spec(body=_maxx(_Src0 * _C0, _Src1 * _C1))
    row = max(_dve_ops._SUB_OPCODE_FOR_NAME.values()) + 1
    assert row < 0x20
    _dve_ops._SUB_OPCODE_FOR_NAME[name] = row
    shas = {}
    for ver in ("v3", "v4"):
        try:
            r = _DveOpSpec(name=name, opcode=row,
                           uops=_dve_lower(spec, ver=ver), rd1_en=True)
            shas[ver] = r.sha(ver)
        except Exception:
            pass
    op = _dve_ops.DveOp(name, spec, subdim=False, uops_sha=shas)
    _dve_ops.OPS.append(op)
    return op


RK1MAX = _register_rk1max()

F32 = mybir.dt.float32
F32R = mybir.dt.float32r
F16 = mybir.dt.float16
BF16 = mybir.dt.bfloat16
AF = mybir.ActivationFunctionType
OP = mybir.AluOpType

N = 4096
INPUT_DIM = 512
H1 = 256
H2 = 64
ALPHA = 0.2
NCORES = 8
NB = N // NCORES          # 512 rows per core
IT = NB // 128            # 4 i-tiles per core
JT = N // 128             # 32 j-tiles

# G (layer-0 gathered rhs) column layout, width 136:
#   0:64 u1 | 64:128 u2 | 128 p1a | 129 p1b | 130 p2a | 131 p2b
#   | 132 ones | 133 B0 | 134 D0 | 135 pad
GW = 136
# G1 (layers 1+2 gathered rhs) column layout, width 136:
#   0:64 h1 | 64 ones | 65 B1 | 66 D1 | 67:131 h2 | 131 ones
#   | 132 B2 | 133 D2 | 134:136 pad
G1W = 136

def build_program():
    nc = bacc.Bacc("TRN2", target_bir_lowering=False, debug=False,
                   num_devices=NCORES)

    # full X^T (all 4096 nodes), host-rearranged to [128, 4, N]: every core
    # builds the complete layer-0 rhs G locally (redundantly) instead of
    # gathering it — kills the first AllGather and its skew barrier.
    xt = nc.dram_tensor("xt", [128, 4, N], F16, kind="ExternalInput").ap()
    # wbig cols: 0:132 attention rhs (u1|u2|p1a|p1b|p2a|p2b), 132 = f2 weight
    wbig = nc.dram_tensor("wbig", [128, 4, 133], F16,
                          kind="ExternalInput").ap()
    # own rows of X^T + f1 weight column (for the A/C broadcast rows)
    xto = nc.dram_tensor("xto", [128, 4, NB], F16, kind="ExternalInput").ap()
    wfo = nc.dram_tensor("wfo", [128, 4, 1], F16, kind="ExternalInput").ap()
    maskT = nc.dram_tensor("maskT", [N, NB], BF16, kind="ExternalInput").ap()
    noiseT = nc.dram_tensor("noiseT", [H2, NB], F32, kind="ExternalInput").ap()
    apred = nc.dram_tensor("apred", [NB, N], F32, kind="ExternalOutput").ap()

    rg = [list(range(NCORES))]

    with tile.TileContext(nc) as tc, \
         tc.tile_pool(name="perm", bufs=1) as perm, \
         tc.tile_pool(name="gdram", bufs=1, space="DRAM") as gdram:

        # ---------- long-lived tiles ----------
        ident = perm.tile([128, 128], F32)
        make_identity(nc, ident)
        ones1 = perm.tile([1, 128], BF16)
        nc.vector.memset(ones1, 1.0)
        onesr = perm.tile([1, 64], F32R)
        ones64f = perm.tile([1, 64], F32)
        nc.vector.memset(ones64f, 1.0)
        nc.scalar.activation(onesr, ones64f, AF.Copy)

        mask_g = [perm.tile([128, 4, NB], BF16, tag=f"maskg{g}",
                             name=f"maskg{g}") for g in range(JT // 4)]

        bc0a = perm.tile([128, NB], BF16)
        bc0c = perm.tile([128, NB], BF16)
        bc1a = perm.tile([128, NB], BF16)
        bc1c = perm.tile([128, NB], BF16)
        bc2a = perm.tile([128, NB], BF16)
        bc2c = perm.tile([128, NB], BF16)
        bd0 = perm.tile([128, JT, 2], F32)           # f32 B0/D0 scalar cols
        bd2 = perm.tile([128, JT, 2], F32)           # f32 B/D cols layer 2
        bd1 = perm.tile([128, JT, 2], F32)           # f32 B/D cols layer 1
        NQ = JT // 4
        r0q = [perm.tile([128, NQ, GW], BF16, tag=f"r0q{q}", name=f"r0q{q}")
               for q in range(4)]
        # g1 is gathered in two column halves (logstd cols first) so the
        # layer-2 score loop overlaps the second half's AllGather.
        r1qa = [perm.tile([128, NQ, 68], BF16, tag=f"r1qa{q}",
                          name=f"r1qa{q}") for q in range(4)]
        r1qb = [perm.tile([128, NQ, 68], BF16, tag=f"r1qb{q}",
                          name=f"r1qb{q}") for q in range(4)]
        noiseT_sb = perm.tile([64, NB], F32)
        nc.sync.dma_start(out=noiseT_sb, in_=noiseT)
        zt_own = perm.tile([64, NB], F16)
        ztb = perm.tile([64, NCORES, NB], F16)

        g1a_in = gdram.tile([NB, 68], BF16)
        g1a_out = gdram.tile([N, 68], BF16, addr_space="Shared")
        g1b_in = gdram.tile([NB, 68], BF16)
        g1b_out = gdram.tile([N, 68], BF16, addr_space="Shared")
        ztg_in = gdram.tile([64, NB], F16)
        ztg_out = gdram.tile([NCORES * 64, NB], F16, addr_space="Shared")

        # ---------------- stage A: build FULL G = [X @ Wbig | exps] -------
        # Every core computes all 4096 rows of the layer-0 attention rhs
        # locally (redundant ~15us of PE) — no AllGather, no skew barrier.
        with tc.tile_pool(name="bld_sb", bufs=2) as bsb, \
             tc.tile_pool(name="bld_ps", bufs=2, space="PSUM") as bps:

            NXC = 4                      # X^T DMA chunks (pipelining)
            XW = N // NXC                # 1024 nodes per chunk
            TPC = JT // NXC              # 8 j-tiles per chunk
            wb_sb = bsb.tile([128, 4, 133], F16, tag="wb_sb", bufs=1)
            nc.sync.dma_start(out=wb_sb, in_=wbig)
            wf_sb = bsb.tile([128, 4, 1], F16, tag="wf_sb", bufs=1)
            nc.sync.dma_start(out=wf_sb, in_=wfo)
            xto_sb = bsb.tile([128, 4, NB], F16, tag="xto_sb", bufs=1)
            nc.sync.dma_start(out=xto_sb, in_=xto)
            xt_sb = [bsb.tile([128, 4, XW], F16, tag=f"xt{q}",
                              name=f"xt{q}", bufs=1) for q in range(NXC)]
            for q in range(NXC):
                nc.sync.dma_start(out=xt_sb[q],
                                  in_=xt[:, :, q * XW:(q + 1) * XW])
            for g in range(JT // 4):
                nc.sync.dma_start(
                    out=mask_g[g],
                    in_=maskT[g * 512:(g + 1) * 512, :]
                    .rearrange("(t p) i -> p t i", p=128))

            a0row = bsb.tile([1, NB], BF16, tag="a0row", bufs=1)
            c0row = bsb.tile([1, NB], BF16, tag="c0row", bufs=1)

            # own f1 -> exp'd A0/C0 rows, then broadcast tiles
            for s in range(IT):
                psF = bps.tile([128, 1], F32, tag="psF")
                for k in range(4):
                    nc.tensor.matmul(psF, lhsT=xto_sb[:, k, ts(s, 128)],
                                     rhs=wf_sb[:, k, :],
                                     start=(k == 0), stop=(k == 3))
                fcol = bsb.tile([128, 1], F32, tag="fcol")
                nc.scalar.activation(fcol, psF, AF.Copy)
                psT = bps.tile([1, 128], F32, tag="psT")
                nc.tensor.transpose(psT, fcol, ident)
                nc.scalar.activation(a0row[0:1, ts(s, 128)], psT, AF.Exp)
                nc.scalar.activation(c0row[0:1, ts(s, 128)], psT, AF.Exp,
                                     scale=ALPHA)
            for dst, row in ((bc0a, a0row), (bc0c, c0row)):
                psB = bps.tile([128, NB], F32, tag="psB")
                nc.tensor.matmul(psB, lhsT=ones1, rhs=row, start=True,
                                 stop=True)
                nc.scalar.activation(dst, psB, AF.Copy)

            # full G rows, built straight into the r0q consumption layout
            for t in range(JT):
                q, r = t // NQ, t % NQ
                psA = bps.tile([128, 133], F32, tag="psA")
                xs = xt_sb[t // TPC]
                for k in range(4):
                    nc.tensor.matmul(
                        psA, lhsT=xs[:, k, ts(t % TPC, 128)],
                        rhs=wb_sb[:, k, :], start=(k == 0), stop=(k == 3))
                nc.vector.tensor_copy(r0q[q][:, r, 0:132], psA[:, 0:132])
                nc.vector.memset(r0q[q][:, r, 132:133], 1.0)
                nc.scalar.activation(r0q[q][:, r, 133:134], psA[:, 132:133],
                                     AF.Exp)
                nc.scalar.activation(r0q[q][:, r, 134:135], psA[:, 132:133],
                                     AF.Exp, scale=ALPHA)
                nc.vector.memset(r0q[q][:, r, 135:136], 0.0)

            for q in range(4):
                nc.vector.tensor_copy(bd0[:, q * NQ:(q + 1) * NQ, :],
                                      r0q[q][:, :, 133:135])

        # ---------------- stage C: layer-0 attention pass ------------------
        with tc.tile_pool(name="p0_ps", bufs=1, space="PSUM") as p0ps, \
             tc.tile_pool(name="p0_v", bufs=5) as vp:

            ps0 = [p0ps.tile([128, 133], F32, tag=f"ps0_{s}",
                             name=f"ps0_{s}") for s in range(IT)]
            # HAM warmup: ~4.5us of fp32 matmuls, gated on gathered data so
            # they run right before the real pass-0 matmuls and flip the PE
            # clock gate to 2.4 GHz (the pass itself never sustains 3.4us of
            # continuous PE busy, so it would otherwise run cold forever).
            psW = p0ps.tile([128, 128], F32, tag="psW")
            nc.tensor.matmul(psW[:, 0:64], lhsT=ident,
                             rhs=bd0[:, 0:JT, :].rearrange("p t c -> p (t c)"),
                             start=True, stop=True)
            for w in range(10):
                nc.tensor.matmul(psW, lhsT=ident, rhs=ident,
                                 start=True, stop=True)
            for g in range(JT // 4):
                t3s = vp.tile([128, 4, NB], BF16, tag="t3s", name=f"t3s0_{g}")
                for u in range(4):
                    t = 4 * g + u
                    if t % 9 < 4:
                        t1 = vp.tile([128, NB], BF16, tag="t1")
                        nc.scalar.activation(t1, bc0a, AF.Copy,
                                             scale=bd0[:, t, 0:1])
                        t2 = vp.tile([128, NB], BF16, tag="t2")
                        nc.scalar.activation(t2, bc0c, AF.Copy,
                                             scale=bd0[:, t, 1:2])
                        nc.vector.tensor_tensor(t3s[:, u, :], t1, t2,
                                                op=OP.max)
                    else:
                        nc.vector._custom_dve(
                            RK1MAX, out=t3s[:, u, :], in0=bc0a, in1=bc0c,
                            s0=bd0[:, t, 0:1], s1=bd0[:, t, 1:2])
                pts = vp.tile([128, 4, NB], BF16, tag="pts", name=f"pts0_{g}")
                nc.vector.tensor_tensor(pts, t3s, mask_g[g], op=OP.mult)
                # keep the PE clock gate warm through the VE-paced loop
                nc.tensor.matmul(psW, lhsT=ident, rhs=ident,
                                 start=True, stop=True)
                nc.tensor.matmul(psW, lhsT=ident, rhs=ident,
                                 start=True, stop=True)
                for u in range(4):
                    t = 4 * g + u
                    for s in range(IT):
                        nc.tensor.matmul(ps0[s],
                                         lhsT=pts[:, u, ts(s, 128)],
                                         rhs=r0q[t // NQ][:, t % NQ, 0:133],
                                         start=(t == 0), stop=(t == JT - 1))

            # ---------------- stage D: normalize + build G1 ----------------
            with tc.tile_pool(name="d_sb", bufs=2) as dsb, \
                 tc.tile_pool(name="d_ps", bufs=1, space="PSUM") as dps:

                rows12 = dsb.tile([1, 4, NB], BF16, tag="rows12", bufs=1)

                r0cs = []
                for s in range(IT):
                    r0c = dsb.tile([128, 1], F32, tag=f"r0c{s}",
                                   name=f"r0c{s}")
                    nc.vector.reciprocal(r0c, ps0[s][:, 132:133])
                    r0a = dsb.tile([128, 1], F32, tag=f"r0a{s}",
                                   name=f"r0a{s}")
                    nc.vector.tensor_scalar_mul(r0a, r0c, ALPHA)
                    r0cs.append((r0c, r0a))

                    # cols 0:68 = logstd half (h2|ones|B2|D2|pad), gathered
                    # first; cols 68:136 = mean half (h1|ones|B1|D1|pad)
                    g1own = dsb.tile([128, G1W], BF16, tag="g1own")
                    nc.vector.tensor_scalar_mul(g1own[:, 0:64],
                                                ps0[s][:, 64:128], r0c)
                    nc.vector.memset(g1own[:, 64:65], 1.0)
                    nc.scalar.activation(g1own[:, 65:66], ps0[s][:, 131:132],
                                         AF.Exp, scale=r0c)
                    nc.scalar.activation(g1own[:, 66:67], ps0[s][:, 131:132],
                                         AF.Exp, scale=r0a)
                    nc.vector.memset(g1own[:, 67:68], 0.0)
                    nc.vector.tensor_scalar_mul(g1own[:, 68:132],
                                                ps0[s][:, 0:64], r0c)
                    nc.vector.memset(g1own[:, 132:133], 1.0)
                    nc.scalar.activation(g1own[:, 133:134], ps0[s][:, 129:130],
                                         AF.Exp, scale=r0c)
                    nc.scalar.activation(g1own[:, 134:135], ps0[s][:, 129:130],
                                         AF.Exp, scale=r0a)
                    nc.vector.memset(g1own[:, 135:136], 0.0)
                    nc.sync.dma_start(out=g1a_in[ts(s, 128), :],
                                      in_=g1own[:, 0:68])
                    nc.sync.dma_start(out=g1b_in[ts(s, 128), :],
                                      in_=g1own[:, 68:136])

                nc.gpsimd.collective_compute(
                    "AllGather", OP.bypass, replica_groups=rg,
                    ins=[g1a_in.opt()], outs=[g1a_out.opt()])
                nc.gpsimd.collective_compute(
                    "AllGather", OP.bypass, replica_groups=rg,
                    ins=[g1b_in.opt()], outs=[g1b_out.opt()])

                # f1' (col 128) and f1'' (col 130) -> exp'd rows; runs on
                # ACT/PE while the gather is in flight
                for s in range(IT):
                    r0c, _ = r0cs[s]
                    for li, col in ((0, 128), (2, 130)):
                        fcl = dsb.tile([128, 1], F32, tag="fcl")
                        nc.scalar.activation(fcl, ps0[s][:, col:col + 1],
                                             AF.Copy, scale=r0c)
                        psT2 = dps.tile([1, 128], F32, tag="psT2")
                        nc.tensor.transpose(psT2, fcl, ident)
                        nc.scalar.activation(rows12[0:1, li, ts(s, 128)],
                                             psT2, AF.Exp)
                        nc.scalar.activation(rows12[0:1, li + 1, ts(s, 128)],
                                             psT2, AF.Exp, scale=ALPHA)

                for i, dst in enumerate((bc1a, bc1c, bc2a, bc2c)):
                    psB2 = dps.tile([128, NB], F32, tag="psB2")
                    nc.tensor.matmul(psB2, lhsT=ones1,
                                     rhs=rows12[0:1, i, :], start=True,
                                     stop=True)
                    nc.scalar.activation(dst, psB2, AF.Copy)

                for q in range(4):
                    nc.sync.dma_start(
                        out=r1qa[q],
                        in_=g1a_out[q * NQ * 128:(q + 1) * NQ * 128, :]
                        .rearrange("(t p) c -> p t c", p=128))
                    nc.vector.tensor_copy(bd2[:, q * NQ:(q + 1) * NQ, :],
                                          r1qa[q][:, :, 65:67])
                for q in range(4):
                    nc.sync.dma_start(
                        out=r1qb[q],
                        in_=g1b_out[q * NQ * 128:(q + 1) * NQ * 128, :]
                        .rearrange("(t p) c -> p t c", p=128))
                    nc.vector.tensor_copy(bd1[:, q * NQ:(q + 1) * NQ, :],
                                          r1qb[q][:, :, 65:67])

        # -------- stage E: layers 1+2, interleaved, transposed outputs -----
        # psT[c, i] = sum_j G1[j, c] * P[j, i]; row 64 = denominator.
        with tc.tile_pool(name="e_ps", bufs=1, space="PSUM") as eps, \
             tc.tile_pool(name="e_v", bufs=5) as vpl, \
             tc.tile_pool(name="e_sb", bufs=1) as esb:

            ps1T = eps.tile([65, NB], F32, tag="ps1T")
            ps2T = eps.tile([65, NB], F32, tag="ps2T")
            psW2 = eps.tile([128, 128], F32, tag="psW2")
            nc.tensor.matmul(psW2[:, 0:64], lhsT=ident,
                             rhs=bd2[:, 0:JT, :]
                             .rearrange("p t c -> p (t c)"),
                             start=True, stop=True)
            for w in range(10):
                nc.tensor.matmul(psW2, lhsT=ident, rhs=ident,
                                 start=True, stop=True)

            def p_group(g, uniq, bca, bcc, bd, pool):
                t3s = pool.tile([128, 4, NB], BF16, tag="t3s",
                                name=f"t3se_{uniq}_{g}")
                for u in range(4):
                    t = 4 * g + u
                    if (t + 2 * uniq) % 15 < 8:
                        t1 = pool.tile([128, NB], BF16, tag="t1",
                                       name=f"t1e_{uniq}_{t}")
                        nc.scalar.activation(t1, bca, AF.Copy,
                                             scale=bd[:, t, 0:1])
                        t2 = pool.tile([128, NB], BF16, tag="t2",
                                       name=f"t2e_{uniq}_{t}")
                        nc.scalar.activation(t2, bcc, AF.Copy,
                                             scale=bd[:, t, 1:2])
                        nc.vector.tensor_tensor(t3s[:, u, :], t1, t2,
                                                op=OP.max)
                    else:
                        nc.vector._custom_dve(
                            RK1MAX, out=t3s[:, u, :], in0=bca, in1=bcc,
                            s0=bd[:, t, 0:1], s1=bd[:, t, 1:2])
                pts = pool.tile([128, 4, NB], BF16, tag="pts",
                                name=f"ptse_{uniq}_{g}")
                nc.vector.tensor_tensor(pts, t3s, mask_g[g], op=OP.mult)
                if g % 4 == 3:
                    nc.tensor.matmul(psW2, lhsT=ident, rhs=ident,
                                     start=True, stop=True)
                    nc.tensor.matmul(psW2, lhsT=ident, rhs=ident,
                                     start=True, stop=True)
                return pts

            # pass 2 (logstd) first so its Z-chain overlaps pass 1 and the
            # whole loop overlaps the mean half's AllGather
            for g in range(JT // 4):
                pts = p_group(g, 2, bc2a, bc2c, bd2, vpl)
                for u in range(4):
                    t = 4 * g + u
                    nc.tensor.matmul(ps2T,
                                     lhsT=r1qa[t // NQ][:, t % NQ, 0:65],
                                     rhs=pts[:, u, :],
                                     start=(t == 0), stop=(t == JT - 1))

            r2row = esb.tile([1, NB], F32)
            nc.vector.reciprocal(r2row, ps2T[64:65, :])
            r2r = esb.tile([1, NB], F32R)
            nc.scalar.activation(r2r, r2row, AF.Copy)
            psBC2 = eps.tile([64, NB], F32, tag="psBC2")
            nc.tensor.matmul(psBC2, lhsT=onesr, rhs=r2r, start=True,
                             stop=True)
            r2bc = esb.tile([64, NB], F32)
            nc.scalar.activation(r2bc, psBC2, AF.Copy)
            ltT = esb.tile([64, NB], F32)
            nc.vector.tensor_tensor(ltT, ps2T[0:64, :], r2bc, op=OP.mult)
            eT = esb.tile([64, NB], F32)
            nc.scalar.activation(eT, ltT, AF.Exp)
            zmT = esb.tile([64, NB], F32)
            nc.vector.tensor_tensor(zmT, eT, noiseT_sb, op=OP.mult)

            for g in range(JT // 4):
                pts = p_group(g, 1, bc1a, bc1c, bd1, vpl)
                for u in range(4):
                    t = 4 * g + u
                    nc.tensor.matmul(ps1T,
                                     lhsT=r1qb[t // NQ][:, t % NQ, 0:65],
                                     rhs=pts[:, u, :],
                                     start=(t == 0), stop=(t == JT - 1))

            r1row = esb.tile([1, NB], F32)
            nc.vector.reciprocal(r1row, ps1T[64:65, :])
            r1r = esb.tile([1, NB], F32R)
            nc.scalar.activation(r1r, r1row, AF.Copy)
            psBC1 = eps.tile([64, NB], F32, tag="psBC1")
            nc.tensor.matmul(psBC1, lhsT=onesr, rhs=r1r, start=True,
                             stop=True)
            r1bc = esb.tile([64, NB], F32)
            nc.scalar.activation(r1bc, psBC1, AF.Copy)
            meanT = esb.tile([64, NB], F32)
            nc.vector.tensor_tensor(meanT, ps1T[0:64, :], r1bc, op=OP.mult)
            zT = esb.tile([64, NB], F32)
            nc.vector.tensor_tensor(zT, zmT, meanT, op=OP.add)
            nc.scalar.activation(zt_own, zT, AF.Copy)

        # ---------------- stage F: gather Z^T -----------------------------
        nc.sync.dma_start(out=ztg_in, in_=zt_own)
        nc.gpsimd.collective_compute(
            "AllGather", OP.bypass, replica_groups=rg,
            ins=[ztg_in.opt()], outs=[ztg_out.opt()])
        nc.sync.dma_start(
            out=ztb, in_=ztg_out.rearrange("(b p) i -> p b i", p=64))

        # ---------------- stage G: decoder sigmoid(Z @ Z^T) ----------------
        with tc.tile_pool(name="dec_ps", bufs=3, space="PSUM") as decps, \
             tc.tile_pool(name="dec_sb", bufs=3) as decsb:
            # own (diagonal) blocks first — busy-work overlapping the Z^T
            # gather (results unused; the main loop rewrites these columns,
            # and skipping the 1MB scratch DMA keeps the decoder phase at
            # the apred-output DMA roofline)
            for s in range(IT):
                psD = decps.tile([128, NB], F32, tag="psDd",
                                 name=f"psDd_{s}", bufs=1)
                nc.tensor.matmul(psD, lhsT=zt_own[:, ts(s, 128)],
                                 rhs=zt_own, start=True, stop=True)
                osb = decsb.tile([128, NB], F32, tag="osbd",
                                 name=f"osbd_{s}", bufs=1)
                nc.scalar.activation(osb, psD, AF.Sigmoid)

            # warm the PE clock gate for the decoder burst (gated on the
            # gathered Z^T so it runs right as the real matmuls unblock)
            psWd = decps.tile([128, NB], F32, tag="psWd", bufs=1)
            for w in range(6):
                nc.tensor.matmul(psWd, lhsT=ztb[:, 0, 0:128],
                                 rhs=ztb[:, w % 2, :], start=True, stop=True)

            # paired j-blocks: 2 matmuls into one 2-bank PSUM tile, then a
            # single [128, 1024] sigmoid and a single contiguous DMA out
            for s in range(IT):
                for bp in range(NCORES // 2):
                    psD2 = decps.tile([128, 2, NB], F32, tag="psD2",
                                      name=f"psD2_{s}_{bp}")
                    nc.tensor.matmul(psD2[:, 0, :],
                                     lhsT=zt_own[:, ts(s, 128)],
                                     rhs=ztb[:, 2 * bp, :],
                                     start=True, stop=True)
                    nc.tensor.matmul(psD2[:, 1, :],
                                     lhsT=zt_own[:, ts(s, 128)],
                                     rhs=ztb[:, 2 * bp + 1, :],
                                     start=True, stop=True)
                    osb2 = decsb.tile([128, 2, NB], F32, tag="osb2",
                                      name=f"osb2_{s}_{bp}", bufs=6)
                    nc.scalar.activation(osb2, psD2, AF.Sigmoid)
                    eng = nc.sync if bp % 2 == 0 else nc.gpsimd
                    eng.dma_start(
                        out=apred[ts(s, 128), ts(bp, 2 * NB)], in_=osb2)

    nc.compile()
    return nc


_program = None


def _get_program():
    global _program
    if _program is None:
        _program = build_program()
    return _program


def kernel(X, adj, noise, W0, a0, W1, a1, W2, a2, _trace=False):
    X = np.asarray(X, dtype=np.float32)
    adj = np.asarray(adj)
    noise = np.asarray(noise, dtype=np.float32)
    W0 = np.asarray(W0, dtype=np.float32)
    a0 = np.asarray(a0, dtype=np.float32)
    W1 = np.asarray(W1, dtype=np.float32)
    a1 = np.asarray(a1, dtype=np.float32)
    W2 = np.asarray(W2, dtype=np.float32)
    a2 = np.asarray(a2, dtype=np.float32)

    # folded weight matrix [512, 133]: attention rhs cols + f2 weight
    u1 = W0 @ W1
    u2 = W0 @ W2
    wbig = np.concatenate([
        u1, u2,
        u1 @ a1[:H2], u1 @ a1[H2:],
        u2 @ a2[:H2], u2 @ a2[H2:],
        W0 @ a0[H1:],
    ], axis=1).astype(np.float32)
    wfo = (W0 @ a0[:H1]).astype(np.float32)  # [512, 1] f1 weight

    maskT = adj.astype(ml_dtypes.bfloat16).T  # 0/1, exact in bf16

    def rearr(m):
        # [512, c] -> [128, 4, c] matching the device-side k-split
        c = m.shape[1]
        return np.ascontiguousarray(
            m.reshape(4, 128, c).transpose(1, 0, 2)).astype(np.float16)

    xt_full = rearr(np.ascontiguousarray(X.T))     # [128, 4, 4096], shared
    wbig_r = rearr(wbig)
    wfo_r = rearr(wfo)

    in_maps = []
    for k in range(NCORES):
        sl = slice(k * NB, (k + 1) * NB)
        in_maps.append({
            "xt": xt_full,
            "wbig": wbig_r,
            "xto": rearr(np.ascontiguousarray(X[sl].T)),
            "wfo": wfo_r,
            "maskT": np.ascontiguousarray(maskT[:, sl]),
            "noiseT": np.ascontiguousarray(noise[sl].T),
        })

    nc = _get_program()
    res = run_bass_kernel_spmd(nc, in_maps, core_ids=list(range(NCORES)),
                               trace=_trace)
    out = np.concatenate([res.results[k]["apred"] for k in range(NCORES)],
                         axis=0)
    if _trace:
        kernel.last_results = res
    return out



# revision 16
# speedup vs baseline: 1.0239x; 1.0239x over previous
"""Trainium2 Bass kernel for nn_AVGAE (3-layer GAT variational graph
autoencoder, N=4096) on 8 NeuronCores.

Sharding: 1D row partition of the N x N attention/score matrices — core k
owns output rows [512k, 512k+512). Small per-node features are all-gathered
between layers (AllGather over internal DRAM tiles).

Key algebraic restructuring (no elementwise transcendentals over N x N):
  exp(leaky_relu(f1_i + f2_j, a)) = max(A_i*B_j, C_i*D_j)
  with A=exp(f1), B=exp(f2), C=exp(a*f1), D=exp(a*f2)
so each N x N score tile is built with vector ALU ops only (outer-product
scalar muls + max + mask mul), all bf16, then consumed directly by the
tensor engine as attention weights.  Softmax denominators come for free as
a ones-column in the attention rhs (exp(MASK_VAL) == 0 exactly in fp32, so
masked entries contribute 0 to numerator and denominator, matching the
reference softmax).

All per-node "h" quantities of layers 1/2 are linear images of layer-0
attention output, so the layer-0 attention rhs carries
[h0@W1 | h0@W2 | per-layer score vectors | ones] and hidden itself is never
materialized.  Host precomputes the folded weight matrix.  Every core
builds the FULL 4096-row layer-0 rhs G locally from the full X (f16) —
~15us of redundant PE beats the ~25us fixed latency + skew barrier of an
AllGather (collectives here cost ~25us regardless of payload size, so the
kernel keeps exactly two: the g1 feature gather and the Z^T gather).

Layers 1 and 2 produce TRANSPOSED outputs (lhsT = the small rhs columns,
moving operand = the P tile, N=512): one matmul per (j-tile, layer) instead
of four, one PSUM bank each, and Z is produced directly in [H2, node]
layout for the fp16 Z Z^T decoder (no transposes).

Engine balance per j-tile (tuned from perfetto traces): a custom fused DVE
op RK1MAX (out = max(in0*s0, in1*s1)) builds the score tile in one VectorE
instruction for ~5/9 of tiles; the rest use two ScalarE copy-with-scale ops
plus a VectorE max.  The mask multiply runs as one VectorE op per 8 j-tiles
([128, 8*512] supertile).  GpSimd is deliberately unused for elementwise
work: its ops are ~8x slower and its SBUF-port contention slows concurrent
VectorE ops ~4x.  Short bursts of dummy fp32 matmuls (gated on gathered
data) keep the PE clock-gate (HAM) at 2.4 GHz through the VE-paced loops.
"""

import numpy as np
import ml_dtypes

import concourse.bass as bass
import concourse.mybir as mybir
import concourse.tile as tile
from concourse import bacc
from concourse.bass import ts
from concourse.bass_utils import run_bass_kernel_spmd
from concourse.masks import make_identity

import concourse.dve_ops as _dve_ops
from concourse.dve_spec import Spec as _Spec, Src0 as _Src0, Src1 as _Src1, \
    C0 as _C0, C1 as _C1, maxx as _maxx, lower as _dve_lower
from concourse.dve_uop import DveOpSpec as _DveOpSpec


def _register_rk1max():
    """Custom fused DVE op: out = max(in0*s0, in1*s1) — builds an attention
    P-tile precursor in one VectorE instruction instead of
    (scalar-mul + scalar_tensor_tensor)."""
    name = "RK1MAX"
    if name in _dve_ops._SUB_OPCODE_FOR_NAME:
        return next(o for o in _dve_ops.OPS if o.name == name)
    spec = _Spec(body=_maxx(_Src0 * _C0, _Src1 * _C1))
    row = max(_dve_ops._SUB_OPCODE_FOR_NAME.values()) + 1
    assert row < 0x20
    _dve_ops._SUB_OPCODE_FOR_NAME[name] = row
    shas = {}
    for ver in ("v3", "v4"):
        try:
            r = _DveOpSpec(name=name, opcode=row,
                           uops=_dve_lower(spec, ver=ver), rd1_en=True)
            shas[ver] = r.sha(ver)
        except Exception:
            pass
    op = _dve_ops.DveOp(name, spec, subdim=False, uops_sha=shas)
    _dve_ops.OPS.append(op)
    return op


RK1MAX = _register_rk1max()

F32 = mybir.dt.float32
F32R = mybir.dt.float32r
F16 = mybir.dt.float16
BF16 = mybir.dt.bfloat16
AF = mybir.ActivationFunctionType
OP = mybir.AluOpType

N = 4096
INPUT_DIM = 512
H1 = 256
H2 = 64
ALPHA = 0.2
NCORES = 8
NB = N // NCORES          # 512 rows per core
IT = NB // 128            # 4 i-tiles per core
JT = N // 128             # 32 j-tiles

# G (layer-0 gathered rhs) column layout, width 136:
#   0:64 u1 | 64:128 u2 | 128 p1a | 129 p1b | 130 p2a | 131 p2b
#   | 132 ones | 133 B0 | 134 D0 | 135 pad
GW = 136
# G1 (layers 1+2 gathered rhs) column layout, width 136:
#   0:64 h1 | 64 ones | 65 B1 | 66 D1 | 67:131 h2 | 131 ones
#   | 132 B2 | 133 D2 | 134:136 pad
G1W = 136

def build_program():
    nc = bacc.Bacc("TRN2", target_bir_lowering=False, debug=False,
                   num_devices=NCORES)

    # full X^T (all 4096 nodes), host-rearranged to [128, 4, N]: every core
    # builds the complete layer-0 rhs G locally (redundantly) instead of
    # gathering it — kills the first AllGather and its skew barrier.
    xt = nc.dram_tensor("xt", [128, 4, N], F16, kind="ExternalInput").ap()
    # wbig cols: 0:132 attention rhs (u1|u2|p1a|p1b|p2a|p2b), 132 = f2 weight
    wbig = nc.dram_tensor("wbig", [128, 4, 133], F16,
                          kind="ExternalInput").ap()
    # own rows of X^T + f1 weight column (for the A/C broadcast rows)
    xto = nc.dram_tensor("xto", [128, 4, NB], F16, kind="ExternalInput").ap()
    wfo = nc.dram_tensor("wfo", [128, 4, 1], F16, kind="ExternalInput").ap()
    maskT = nc.dram_tensor("maskT", [N, NB], BF16, kind="ExternalInput").ap()
    noiseT = nc.dram_tensor("noiseT", [H2, NB], F32, kind="ExternalInput").ap()
    apred = nc.dram_tensor("apred", [NB, N], F32, kind="ExternalOutput").ap()

    rg = [list(range(NCORES))]

    with tile.TileContext(nc) as tc, \
         tc.tile_pool(name="perm", bufs=1) as perm, \
         tc.tile_pool(name="gdram", bufs=1, space="DRAM") as gdram:

        # ---------- long-lived tiles ----------
        ident = perm.tile([128, 128], F32)
        make_identity(nc, ident)
        ones1 = perm.tile([1, 128], BF16)
        nc.vector.memset(ones1, 1.0)
        onesr = perm.tile([1, 64], F32R)
        ones64f = perm.tile([1, 64], F32)
        nc.vector.memset(ones64f, 1.0)
        nc.scalar.activation(onesr, ones64f, AF.Copy)

        mask_g = [perm.tile([128, 4, NB], BF16, tag=f"maskg{g}",
                             name=f"maskg{g}") for g in range(JT // 4)]

        bc0a = perm.tile([128, NB], BF16)
        bc0c = perm.tile([128, NB], BF16)
        bc1a = perm.tile([128, NB], BF16)
        bc1c = perm.tile([128, NB], BF16)
        bc2a = perm.tile([128, NB], BF16)
        bc2c = perm.tile([128, NB], BF16)
        bd0 = perm.tile([128, JT, 2], F32)           # f32 B0/D0 scalar cols
        bd12 = perm.tile([128, 2, JT, 2], F32)       # f32 B/D cols layers 1,2
        NQ = JT // 4
        r0q = [perm.tile([128, NQ, GW], BF16, tag=f"r0q{q}", name=f"r0q{q}")
               for q in range(4)]
        r1q = [perm.tile([128, NQ, G1W], BF16, tag=f"r1q{q}", name=f"r1q{q}")
               for q in range(4)]
        noiseT_sb = perm.tile([64, NB], F32)
        nc.sync.dma_start(out=noiseT_sb, in_=noiseT)
        zt_own = perm.tile([64, NB], F16)
        ztb = perm.tile([64, NCORES, NB], F16)

        g1_in = gdram.tile([NB, G1W], BF16)
        g1_out = gdram.tile([N, G1W], BF16, addr_space="Shared")
        ztg_in = gdram.tile([64, NB], F16)
        ztg_out = gdram.tile([NCORES * 64, NB], F16, addr_space="Shared")

        # ---------------- stage A: build FULL G = [X @ Wbig | exps] -------
        # Every core computes all 4096 rows of the layer-0 attention rhs
        # locally (redundant ~15us of PE) — no AllGather, no skew barrier.
        with tc.tile_pool(name="bld_sb", bufs=2) as bsb, \
             tc.tile_pool(name="bld_ps", bufs=2, space="PSUM") as bps:

            NXC = 4                      # X^T DMA chunks (pipelining)
            XW = N // NXC                # 1024 nodes per chunk
            TPC = JT // NXC              # 8 j-tiles per chunk
            wb_sb = bsb.tile([128, 4, 133], F16, tag="wb_sb", bufs=1)
            nc.sync.dma_start(out=wb_sb, in_=wbig)
            wf_sb = bsb.tile([128, 4, 1], F16, tag="wf_sb", bufs=1)
            nc.sync.dma_start(out=wf_sb, in_=wfo)
            xto_sb = bsb.tile([128, 4, NB], F16, tag="xto_sb", bufs=1)
            nc.sync.dma_start(out=xto_sb, in_=xto)
            xt_sb = [bsb.tile([128, 4, XW], F16, tag=f"xt{q}",
                              name=f"xt{q}", bufs=1) for q in range(NXC)]
            for q in range(NXC):
                nc.sync.dma_start(out=xt_sb[q],
                                  in_=xt[:, :, q * XW:(q + 1) * XW])
            for g in range(JT // 4):
                nc.sync.dma_start(
                    out=mask_g[g],
                    in_=maskT[g * 512:(g + 1) * 512, :]
                    .rearrange("(t p) i -> p t i", p=128))

            a0row = bsb.tile([1, NB], BF16, tag="a0row", bufs=1)
            c0row = bsb.tile([1, NB], BF16, tag="c0row", bufs=1)

            # own f1 -> exp'd A0/C0 rows, then broadcast tiles
            for s in range(IT):
                psF = bps.tile([128, 1], F32, tag="psF")
                for k in range(4):
                    nc.tensor.matmul(psF, lhsT=xto_sb[:, k, ts(s, 128)],
                                     rhs=wf_sb[:, k, :],
                                     start=(k == 0), stop=(k == 3))
                fcol = bsb.tile([128, 1], F32, tag="fcol")
                nc.scalar.activation(fcol, psF, AF.Copy)
                psT = bps.tile([1, 128], F32, tag="psT")
                nc.tensor.transpose(psT, fcol, ident)
                nc.scalar.activation(a0row[0:1, ts(s, 128)], psT, AF.Exp)
                nc.scalar.activation(c0row[0:1, ts(s, 128)], psT, AF.Exp,
                                     scale=ALPHA)
            for dst, row in ((bc0a, a0row), (bc0c, c0row)):
                psB = bps.tile([128, NB], F32, tag="psB")
                nc.tensor.matmul(psB, lhsT=ones1, rhs=row, start=True,
                                 stop=True)
                nc.scalar.activation(dst, psB, AF.Copy)

            # full G rows, built straight into the r0q consumption layout
            for t in range(JT):
                q, r = t // NQ, t % NQ
                psA = bps.tile([128, 133], F32, tag="psA")
                xs = xt_sb[t // TPC]
                for k in range(4):
                    nc.tensor.matmul(
                        psA, lhsT=xs[:, k, ts(t % TPC, 128)],
                        rhs=wb_sb[:, k, :], start=(k == 0), stop=(k == 3))
                nc.vector.tensor_copy(r0q[q][:, r, 0:132], psA[:, 0:132])
                nc.vector.memset(r0q[q][:, r, 132:133], 1.0)
                nc.scalar.activation(r0q[q][:, r, 133:134], psA[:, 132:133],
                                     AF.Exp)
                nc.scalar.activation(r0q[q][:, r, 134:135], psA[:, 132:133],
                                     AF.Exp, scale=ALPHA)
                nc.vector.memset(r0q[q][:, r, 135:136], 0.0)

            for q in range(4):
                nc.vector.tensor_copy(bd0[:, q * NQ:(q + 1) * NQ, :],
                                      r0q[q][:, :, 133:135])

        # ---------------- stage C: layer-0 attention pass ------------------
        with tc.tile_pool(name="p0_ps", bufs=1, space="PSUM") as p0ps, \
             tc.tile_pool(name="p0_v", bufs=5) as vp:

            ps0 = [p0ps.tile([128, 133], F32, tag=f"ps0_{s}",
                             name=f"ps0_{s}") for s in range(IT)]
            # HAM warmup: ~4.5us of fp32 matmuls, gated on gathered data so
            # they run right before the real pass-0 matmuls and flip the PE
            # clock gate to 2.4 GHz (the pass itself never sustains 3.4us of
            # continuous PE busy, so it would otherwise run cold forever).
            psW = p0ps.tile([128, 128], F32, tag="psW")
            nc.tensor.matmul(psW[:, 0:64], lhsT=ident,
                             rhs=bd0[:, 0:JT, :].rearrange("p t c -> p (t c)"),
                             start=True, stop=True)
            for w in range(10):
                nc.tensor.matmul(psW, lhsT=ident, rhs=ident,
                                 start=True, stop=True)
            for g in range(JT // 4):
                t3s = vp.tile([128, 4, NB], BF16, tag="t3s", name=f"t3s0_{g}")
                for u in range(4):
                    t = 4 * g + u
                    if t % 9 < 4:
                        t1 = vp.tile([128, NB], BF16, tag="t1")
                        nc.scalar.activation(t1, bc0a, AF.Copy,
                                             scale=bd0[:, t, 0:1])
                        t2 = vp.tile([128, NB], BF16, tag="t2")
                        nc.scalar.activation(t2, bc0c, AF.Copy,
                                             scale=bd0[:, t, 1:2])
                        nc.vector.tensor_tensor(t3s[:, u, :], t1, t2,
                                                op=OP.max)
                    else:
                        nc.vector._custom_dve(
                            RK1MAX, out=t3s[:, u, :], in0=bc0a, in1=bc0c,
                            s0=bd0[:, t, 0:1], s1=bd0[:, t, 1:2])
                pts = vp.tile([128, 4, NB], BF16, tag="pts", name=f"pts0_{g}")
                nc.vector.tensor_tensor(pts, t3s, mask_g[g], op=OP.mult)
                # keep the PE clock gate warm through the VE-paced loop
                nc.tensor.matmul(psW, lhsT=ident, rhs=ident,
                                 start=True, stop=True)
                nc.tensor.matmul(psW, lhsT=ident, rhs=ident,
                                 start=True, stop=True)
                for u in range(4):
                    t = 4 * g + u
                    for s in range(IT):
                        nc.tensor.matmul(ps0[s],
                                         lhsT=pts[:, u, ts(s, 128)],
                                         rhs=r0q[t // NQ][:, t % NQ, 0:133],
                                         start=(t == 0), stop=(t == JT - 1))

            # ---------------- stage D: normalize + build G1 ----------------
            with tc.tile_pool(name="d_sb", bufs=2) as dsb, \
                 tc.tile_pool(name="d_ps", bufs=1, space="PSUM") as dps:

                rows12 = dsb.tile([1, 4, NB], BF16, tag="rows12", bufs=1)

                r0cs = []
                for s in range(IT):
                    r0c = dsb.tile([128, 1], F32, tag=f"r0c{s}",
                                   name=f"r0c{s}")
                    nc.vector.reciprocal(r0c, ps0[s][:, 132:133])
                    r0a = dsb.tile([128, 1], F32, tag=f"r0a{s}",
                                   name=f"r0a{s}")
                    nc.vector.tensor_scalar_mul(r0a, r0c, ALPHA)
                    r0cs.append((r0c, r0a))

                    g1own = dsb.tile([128, G1W], BF16, tag="g1own")
                    nc.vector.tensor_scalar_mul(g1own[:, 0:64],
                                                ps0[s][:, 0:64], r0c)
                    nc.vector.memset(g1own[:, 64:65], 1.0)
                    nc.scalar.activation(g1own[:, 65:66], ps0[s][:, 129:130],
                                         AF.Exp, scale=r0c)
                    nc.scalar.activation(g1own[:, 66:67], ps0[s][:, 129:130],
                                         AF.Exp, scale=r0a)
                    nc.vector.tensor_scalar_mul(g1own[:, 67:131],
                                                ps0[s][:, 64:128], r0c)
                    nc.vector.memset(g1own[:, 131:132], 1.0)
                    nc.scalar.activation(g1own[:, 132:133], ps0[s][:, 131:132],
                                         AF.Exp, scale=r0c)
                    nc.scalar.activation(g1own[:, 133:134], ps0[s][:, 131:132],
                                         AF.Exp, scale=r0a)
                    nc.vector.memset(g1own[:, 134:136], 0.0)
                    nc.sync.dma_start(out=g1_in[ts(s, 128), :], in_=g1own)

                nc.gpsimd.collective_compute(
                    "AllGather", OP.bypass, replica_groups=rg,
                    ins=[g1_in.opt()], outs=[g1_out.opt()])

                # f1' (col 128) and f1'' (col 130) -> exp'd rows; runs on
                # ACT/PE while the gather is in flight
                for s in range(IT):
                    r0c, _ = r0cs[s]
                    for li, col in ((0, 128), (2, 130)):
                        fcl = dsb.tile([128, 1], F32, tag="fcl")
                        nc.scalar.activation(fcl, ps0[s][:, col:col + 1],
                                             AF.Copy, scale=r0c)
                        psT2 = dps.tile([1, 128], F32, tag="psT2")
                        nc.tensor.transpose(psT2, fcl, ident)
                        nc.scalar.activation(rows12[0:1, li, ts(s, 128)],
                                             psT2, AF.Exp)
                        nc.scalar.activation(rows12[0:1, li + 1, ts(s, 128)],
                                             psT2, AF.Exp, scale=ALPHA)

                for i, dst in enumerate((bc1a, bc1c, bc2a, bc2c)):
                    psB2 = dps.tile([128, NB], F32, tag="psB2")
                    nc.tensor.matmul(psB2, lhsT=ones1,
                                     rhs=rows12[0:1, i, :], start=True,
                                     stop=True)
                    nc.scalar.activation(dst, psB2, AF.Copy)

                for q in range(4):
                    nc.sync.dma_start(
                        out=r1q[q],
                        in_=g1_out[q * NQ * 128:(q + 1) * NQ * 128, :]
                        .rearrange("(t p) c -> p t c", p=128))
                    nc.vector.tensor_copy(bd12[:, 0, q * NQ:(q + 1) * NQ, :],
                                          r1q[q][:, :, 65:67])
                    nc.vector.tensor_copy(bd12[:, 1, q * NQ:(q + 1) * NQ, :],
                                          r1q[q][:, :, 132:134])

        # -------- stage E: layers 1+2, interleaved, transposed outputs -----
        # psT[c, i] = sum_j G1[j, c] * P[j, i]; row 64 = denominator.
        with tc.tile_pool(name="e_ps", bufs=1, space="PSUM") as eps, \
             tc.tile_pool(name="e_v", bufs=5) as vpl, \
             tc.tile_pool(name="e_sb", bufs=1) as esb:

            ps1T = eps.tile([65, NB], F32, tag="ps1T")
            ps2T = eps.tile([65, NB], F32, tag="ps2T")
            psW2 = eps.tile([128, 128], F32, tag="psW2")
            nc.tensor.matmul(psW2[:, 0:64], lhsT=ident,
                             rhs=bd12[:, 1, 0:JT, :]
                             .rearrange("p t c -> p (t c)"),
                             start=True, stop=True)
            for w in range(10):
                nc.tensor.matmul(psW2, lhsT=ident, rhs=ident,
                                 start=True, stop=True)

            def p_group(g, uniq, bca, bcc, bd, pool):
                t3s = pool.tile([128, 4, NB], BF16, tag="t3s",
                                name=f"t3se_{uniq}_{g}")
                for u in range(4):
                    t = 4 * g + u
                    if (t + 2 * uniq) % 15 < 8:
                        t1 = pool.tile([128, NB], BF16, tag="t1",
                                       name=f"t1e_{uniq}_{t}")
                        nc.scalar.activation(t1, bca, AF.Copy,
                                             scale=bd[:, t, 0:1])
                        t2 = pool.tile([128, NB], BF16, tag="t2",
                                       name=f"t2e_{uniq}_{t}")
                        nc.scalar.activation(t2, bcc, AF.Copy,
                                             scale=bd[:, t, 1:2])
                        nc.vector.tensor_tensor(t3s[:, u, :], t1, t2,
                                                op=OP.max)
                    else:
                        nc.vector._custom_dve(
                            RK1MAX, out=t3s[:, u, :], in0=bca, in1=bcc,
                            s0=bd[:, t, 0:1], s1=bd[:, t, 1:2])
                pts = pool.tile([128, 4, NB], BF16, tag="pts",
                                name=f"ptse_{uniq}_{g}")
                nc.vector.tensor_tensor(pts, t3s, mask_g[g], op=OP.mult)
                if g % 4 == 3:
                    nc.tensor.matmul(psW2, lhsT=ident, rhs=ident,
                                     start=True, stop=True)
                    nc.tensor.matmul(psW2, lhsT=ident, rhs=ident,
                                     start=True, stop=True)
                return pts

            # pass 2 (logstd) first so its Z-chain overlaps pass 1
            for g in range(JT // 4):
                pts = p_group(g, 2, bc2a, bc2c, bd12[:, 1, :, :], vpl)
                for u in range(4):
                    t = 4 * g + u
                    nc.tensor.matmul(ps2T,
                                     lhsT=r1q[t // NQ][:, t % NQ, 67:132],
                                     rhs=pts[:, u, :],
                                     start=(t == 0), stop=(t == JT - 1))

            r2row = esb.tile([1, NB], F32)
            nc.vector.reciprocal(r2row, ps2T[64:65, :])
            r2r = esb.tile([1, NB], F32R)
            nc.scalar.activation(r2r, r2row, AF.Copy)
            psBC2 = eps.tile([64, NB], F32, tag="psBC2")
            nc.tensor.matmul(psBC2, lhsT=onesr, rhs=r2r, start=True,
                             stop=True)
            r2bc = esb.tile([64, NB], F32)
            nc.scalar.activation(r2bc, psBC2, AF.Copy)
            ltT = esb.tile([64, NB], F32)
            nc.vector.tensor_tensor(ltT, ps2T[0:64, :], r2bc, op=OP.mult)
            eT = esb.tile([64, NB], F32)
            nc.scalar.activation(eT, ltT, AF.Exp)
            zmT = esb.tile([64, NB], F32)
            nc.vector.tensor_tensor(zmT, eT, noiseT_sb, op=OP.mult)

            for g in range(JT // 4):
                pts = p_group(g, 1, bc1a, bc1c, bd12[:, 0, :, :], vpl)
                for u in range(4):
                    t = 4 * g + u
                    nc.tensor.matmul(ps1T,
                                     lhsT=r1q[t // NQ][:, t % NQ, 0:65],
                                     rhs=pts[:, u, :],
                                     start=(t == 0), stop=(t == JT - 1))

            r1row = esb.tile([1, NB], F32)
            nc.vector.reciprocal(r1row, ps1T[64:65, :])
            r1r = esb.tile([1, NB], F32R)
            nc.scalar.activation(r1r, r1row, AF.Copy)
            psBC1 = eps.tile([64, NB], F32, tag="psBC1")
            nc.tensor.matmul(psBC1, lhsT=onesr, rhs=r1r, start=True,
                             stop=True)
            r1bc = esb.tile([64, NB], F32)
            nc.scalar.activation(r1bc, psBC1, AF.Copy)
            meanT = esb.tile([64, NB], F32)
            nc.vector.tensor_tensor(meanT, ps1T[0:64, :], r1bc, op=OP.mult)
            zT = esb.tile([64, NB], F32)
            nc.vector.tensor_tensor(zT, zmT, meanT, op=OP.add)
            nc.scalar.activation(zt_own, zT, AF.Copy)

        # ---------------- stage F: gather Z^T -----------------------------
        nc.sync.dma_start(out=ztg_in, in_=zt_own)
        nc.gpsimd.collective_compute(
            "AllGather", OP.bypass, replica_groups=rg,
            ins=[ztg_in.opt()], outs=[ztg_out.opt()])
        nc.sync.dma_start(
            out=ztb, in_=ztg_out.rearrange("(b p) i -> p b i", p=64))

        # ---------------- stage G: decoder sigmoid(Z @ Z^T) ----------------
        with tc.tile_pool(name="dec_ps", bufs=3, space="PSUM") as decps, \
             tc.tile_pool(name="dec_sb", bufs=3) as decsb:
            # own (diagonal) blocks first — overlap with the Z^T gather
            for s in range(IT):
                psD = decps.tile([128, NB], F32, tag="psDd",
                                 name=f"psDd_{s}", bufs=1)
                nc.tensor.matmul(psD, lhsT=zt_own[:, ts(s, 128)],
                                 rhs=zt_own, start=True, stop=True)
                osb = decsb.tile([128, NB], F32, tag="osbd",
                                 name=f"osbd_{s}", bufs=1)
                nc.scalar.activation(osb, psD, AF.Sigmoid)

            # warm the PE clock gate for the decoder burst (gated on the
            # gathered Z^T so it runs right as the real matmuls unblock)
            psWd = decps.tile([128, NB], F32, tag="psWd", bufs=1)
            for w in range(6):
                nc.tensor.matmul(psWd, lhsT=ztb[:, 0, 0:128],
                                 rhs=ztb[:, w % 2, :], start=True, stop=True)

            # paired j-blocks: 2 matmuls into one 2-bank PSUM tile, then a
            # single [128, 1024] sigmoid and a single contiguous DMA out
            for s in range(IT):
                for bp in range(NCORES // 2):
                    psD2 = decps.tile([128, 2, NB], F32, tag="psD2",
                                      name=f"psD2_{s}_{bp}")
                    nc.tensor.matmul(psD2[:, 0, :],
                                     lhsT=zt_own[:, ts(s, 128)],
                                     rhs=ztb[:, 2 * bp, :],
                                     start=True, stop=True)
                    nc.tensor.matmul(psD2[:, 1, :],
                                     lhsT=zt_own[:, ts(s, 128)],
                                     rhs=ztb[:, 2 * bp + 1, :],
                                     start=True, stop=True)
                    osb2 = decsb.tile([128, 2, NB], F32, tag="osb2",
                                      name=f"osb2_{s}_{bp}", bufs=6)
                    nc.scalar.activation(osb2, psD2, AF.Sigmoid)
                    eng = nc.sync if bp % 2 == 0 else nc.gpsimd
                    eng.dma_start(
                        out=apred[ts(s, 128), ts(bp, 2 * NB)], in_=osb2)

    nc.compile()
    return nc


_program = None


def _get_program():
    global _program
    if _program is None:
        _program = build_program()
    return _program


def kernel(X, adj, noise, W0, a0, W1, a1, W2, a2, _trace=False):
    X = np.asarray(X, dtype=np.float32)
    adj = np.asarray(adj)
    noise = np.asarray(noise, dtype=np.float32)
    W0 = np.asarray(W0, dtype=np.float32)
    a0 = np.asarray(a0, dtype=np.float32)
    W1 = np.asarray(W1, dtype=np.float32)
    a1 = np.asarray(a1, dtype=np.float32)
    W2 = np.asarray(W2, dtype=np.float32)
    a2 = np.asarray(a2, dtype=np.float32)

    # folded weight matrix [512, 133]: attention rhs cols + f2 weight
    u1 = W0 @ W1
    u2 = W0 @ W2
    wbig = np.concatenate([
        u1, u2,
        u1 @ a1[:H2], u1 @ a1[H2:],
        u2 @ a2[:H2], u2 @ a2[H2:],
        W0 @ a0[H1:],
    ], axis=1).astype(np.float32)
    wfo = (W0 @ a0[:H1]).astype(np.float32)  # [512, 1] f1 weight

    maskT = adj.astype(ml_dtypes.bfloat16).T  # 0/1, exact in bf16

    def rearr(m):
        # [512, c] -> [128, 4, c] matching the device-side k-split
        c = m.shape[1]
        return np.ascontiguousarray(
            m.reshape(4, 128, c).transpose(1, 0, 2)).astype(np.float16)

    xt_full = rearr(np.ascontiguousarray(X.T))     # [128, 4, 4096], shared
    wbig_r = rearr(wbig)
    wfo_r = rearr(wfo)

    in_maps = []
    for k in range(NCORES):
        sl = slice(k * NB, (k + 1) * NB)
        in_maps.append({
            "xt": xt_full,
            "wbig": wbig_r,
            "xto": rearr(np.ascontiguousarray(X[sl].T)),
            "wfo": wfo_r,
            "maskT": np.ascontiguousarray(maskT[:, sl]),
            "noiseT": np.ascontiguousarray(noise[sl].T),
        })

    nc = _get_program()
    res = run_bass_kernel_spmd(nc, in_maps, core_ids=list(range(NCORES)),
                               trace=_trace)
    out = np.concatenate([res.results[k]["apred"] for k in range(NCORES)],
                         axis=0)
    if _trace:
        kernel.last_results = res
    return out

